# revision 18
# baseline (speedup 1.0000x reference)
"""Fused multi-head-attention (full-width variant) for 8 TRN2 NeuronCores.

Strategy: pure data-parallel over batch (B=8 -> one batch per core).

Algebraic folding (softmax is the only nonlinearity):
  E = (x Wq^T)(x Wk^T)^T * 8 = x M x^T        with M  = 8 * Wq^T Wk  (host fp32)
  y = (P (x Wv^T)) Wo^T      = P (x Mo)       with Mo = Wv^T Wo^T    (host fp32)
so the device only runs TWO projections (q' = x M, v' = x Mo), the energy
matmul against raw xT, and a SPARSE "PV":

The energy rows have std ~ 8*sqrt(768) ~ 222, so softmax(E) is near-one-hot:
virtually all of each row's mass sits on its top couple of entries (top-2
covers it to ~7e-3 relative on the real inputs; fp32-exact top-8 misses at
most 1e-7 of any row's mass beyond rank 8).  Instead of the dense P V matmul
(half the FLOPs of the kernel), each 128-row tile does:
  E tile   [128,2048] fp32 in PSUM (fp16 matmuls, 4 banks)
  DVE max8      -> top-8 values   (fp32 scan; fp16 would tie and drop rows)
  DVE max_index -> top-8 indices
  ACT exp(top8 - max) + accum     -> softmax weights over the top-8
  idxs k=0,1 -> HBM bounce -> [16,2k] int16 layout -> gpsimd dma_gather
  gathered v' rows combined: y = w0*g0 + w1*g1  (ACT mul + DVE STT)
v' is written to HBM right after its projection so gathers can start as soon
as each tile's scan lands.  y is produced in natural [N,E] layout (fp16).

Everything fp16 (fp32 PSUM accumulation), "feature-on-partition" layouts for
the projections, as before.  All input DMAs stream on the SP (sync) HWDGE
queue in exact consumption order; v'-writeback + index bounces share the SP
queue; y/output DMAs go on the ACT queue; gathers ride the gpsimd SWDGE.
"""
import sys

sys.path.insert(0, "/opt/trn_rl_repo")

import numpy as np

import concourse.bass as bass  # noqa: F401
import concourse.tile as tile
from concourse import bacc, library_config, mybir

F32 = mybir.dt.float32
FP16 = mybir.dt.float16
U16 = mybir.dt.uint16
I16 = mybir.dt.int16
AX = mybir.AxisListType.X
MAX = mybir.AluOpType.max
MULT = mybir.AluOpType.mult
ADD = mybir.AluOpType.add

B = 8
E = 768
N = 2048
EC = E // 128      # 6 feature chunks
NT = N // 128      # 16 token chunks / row tiles
NBLK = N // 512    # 4 blocks of 512 tokens
SCALE = 8.0        # sqrt(head_dim); reference multiplies by it
K = 2              # gathered candidates per row

_CACHE = {}

# CoreSim reads gather idxs from partitions 0:16 regardless of queue; real
# HW reads them from the queue's own partition block.  The extra 0:16 copy
# costs a sync-queue DMA per tile, so only emit it for CoreSim runs.
SIM_COMPAT = False


def _build():
    nc = bacc.Bacc("TRN2", target_bir_lowering=False, debug=False, num_devices=B)

    # xT/m come in partition-major layouts so every DMA chunk is contiguous
    # per partition (1.5-6 KB pieces -> full DMA bandwidth at kernel start):
    #   xT[nb, p, c, nn] = x.T[c*128+p, nb*512+nn]
    #   m[dc, p, c, dd]  = M[c*128+p, dc*128+dd]
    xT_d = nc.dram_tensor("xT", [NBLK, 128, EC, 512], FP16, kind="ExternalInput")
    m_d = nc.dram_tensor("m", [EC, 128, E], FP16, kind="ExternalInput")
    mo_d = nc.dram_tensor("mo", [E, E], FP16, kind="ExternalInput")
    y_d = nc.dram_tensor("y", [N, E], FP16, kind="ExternalOutput")
    vh_d = nc.dram_tensor("vh", [N, E], FP16, kind="Internal")   # gather source
    jd_d = nc.dram_tensor("jd", [NT, 128, K], U16, kind="Internal")  # idx bounce

    mo_r = mo_d.rearrange("(c p) f -> p c f", p=128)
    y_r = y_d.rearrange("(t p) e -> p t e", p=128)
    vh_r = vh_d.rearrange("(t p) e -> p t e", p=128)
    # jd_r[t, p, k, c] = jd[t, c*16+p, k]  -> exactly dma_gather's
    # [16, num_idxs/16] wrapped order for flat n = k*128 + i.
    jd_r = jd_d.rearrange("t (c p) k -> t p k c", p=16)

    with tile.TileContext(nc) as tc:
        with tc.tile_pool(name="xT", bufs=1) as xtp, \
             tc.tile_pool(name="qT", bufs=1) as qtp, \
             tc.tile_pool(name="vv", bufs=1) as vvp, \
             tc.tile_pool(name="ix", bufs=1) as ixp:
            xT = xtp.tile([128, NBLK, EC, 512], FP16)   # 24 KB/partition
            qT = qtp.tile([128, EC, N], FP16)   # 24   (q' transposed)
            v = vvp.tile([128, NT, E], FP16)    # 24   (v' natural, HBM staging)
            idx_sb = ixp.tile([128, NT, K, 8], I16)  # gather idxs (parts 0-15)

            # ---------------- stage B: projections ----------------
            with tc.tile_pool(name="wr", bufs=1) as wrp, \
                 tc.tile_pool(name="wp", bufs=2) as wpp, \
                 tc.tile_pool(name="psb", bufs=8, space="PSUM") as psb:
                # PE warm-up during the initial input-DMA window: dummy
                # matmuls push the HAM activity window so the first real
                # matmuls run at 2.4 GHz instead of 1.2 GHz.
                wrm = wrp.tile([128, 512], FP16, tag="wrm")
                nc.gpsimd.memset(wrm[:], 0.0)
                # dma_gather needs the mlp gpsimd library; partitions 16-127
                # of idx_sb are never written by the bounce DMAs but ARE read
                # by the gather -> zero them once.
                nc.gpsimd.memset(idx_sb[:], 0)
                nc.gpsimd.load_library(library_config.mlp)

                def wmm():
                    wps = psb.tile([128, 512], F32, tag="ps")
                    nc.tensor.matmul(
                        wps[:],
                        lhsT=wrm[:, 0:128],
                        rhs=wrm[:],
                        start=True,
                        stop=True,
                    )

                for _w in range(10):
                    wmm()
                m_t = wpp.tile([128, EC, E], FP16, tag="w")  # 9 x2
                mo_t = wpp.tile([128, EC, E], FP16, tag="w")
                # One queue, exact consumption order (parallel HWDGE queues
                # do NOT add early bandwidth — measured twice — they contend
                # and delay the critical chunks). m and xT-nb0 stream in small
                # interleaved chunks; with the e-outer first blocks below,
                # every arriving chunk unlocks ~6 real matmuls.
                nc.sync.dma_start(m_t[:, :, 0:128], m_d[0])
                nc.sync.dma_start(xT[:, 0, 0, :], xT_d[0][:, 0, :])
                for f in range(1, EC):
                    nc.sync.dma_start(m_t[:, :, f * 128:(f + 1) * 128], m_d[f])
                    nc.sync.dma_start(xT[:, 0, f, :], xT_d[0][:, f, :])
                nc.sync.dma_start(xT[:, 1, 0:3, :], xT_d[1][:, 0:3, :])
                nc.sync.dma_start(xT[:, 1, 3:6, :], xT_d[1][:, 3:6, :])
                nc.sync.dma_start(xT[:, 2], xT_d[2])
                nc.sync.dma_start(xT[:, 3], xT_d[3])
                nc.sync.dma_start(mo_t[:], mo_r[:])

                # q'T = M^T @ xT. First block e-outer across 6 concurrent
                # PSUM accumulation groups (paced by the arriving chunks);
                # remaining blocks f-outer (their data streams well ahead).
                ps0 = [
                    psb.tile([128, 512], F32, tag="ps", name=f"ps0_{f}")
                    for f in range(EC)
                ]
                for e in range(EC):
                    for f in range(EC):
                        nc.tensor.matmul(
                            ps0[f][:],
                            lhsT=m_t[:, e, f * 128:(f + 1) * 128],
                            rhs=xT[:, 0, e, :],
                            start=(e == 0),
                            stop=(e == EC - 1),
                        )
                for f in range(EC):
                    nc.vector.tensor_copy(qT[:, f, 0:512], ps0[f][:])
                # nb1 likewise e-outer (its xT block arrives in two halves
                # while nb0's tail is still streaming)
                ps1 = [
                    psb.tile([128, 512], F32, tag="ps", name=f"ps1_{f}")
                    for f in range(EC)
                ]
                for e in range(EC):
                    for f in range(EC):
                        nc.tensor.matmul(
                            ps1[f][:],
                            lhsT=m_t[:, e, f * 128:(f + 1) * 128],
                            rhs=xT[:, 1, e, :],
                            start=(e == 0),
                            stop=(e == EC - 1),
                        )
                for f in range(EC):
                    nc.vector.tensor_copy(qT[:, f, 512:1024], ps1[f][:])
                for nb in range(2, NBLK):
                    for f in range(EC):
                        ps = psb.tile([128, 512], F32, tag="ps")
                        for e in range(EC):
                            nc.tensor.matmul(
                                ps[:],
                                lhsT=m_t[:, e, f * 128:(f + 1) * 128],
                                rhs=xT[:, nb, e, :],
                                start=(e == 0),
                                stop=(e == EC - 1),
                            )
                        nc.vector.tensor_copy(qT[:, f, nb * 512:(nb + 1) * 512], ps[:])

                # v' (natural layout) = x @ Mo; each finished token-chunk is
                # immediately staged to HBM (gather source) on the SP queue.
                for t in range(NT):
                    for flo, fhi in ((0, 512), (512, 768)):
                        ps = psb.tile([128, 512], F32, tag="ps")
                        for e in range(EC):
                            nc.tensor.matmul(
                                ps[:, :fhi - flo],
                                lhsT=xT[:, t // 4, e, (t % 4) * 128:(t % 4 + 1) * 128],
                                rhs=mo_t[:, e, flo:fhi],
                                start=(e == 0),
                                stop=(e == EC - 1),
                            )
                        nc.scalar.copy(v[:, t, flo:fhi], ps[:, :fhi - flo])
                    # ACT queue: keeps Q_I free so the first index bounces
                    # aren't stuck behind 2048 v-writeback descriptors
                    nc.scalar.dma_start(vh_r[:, t, :], v[:, t, :])

            # ---------------- stage C: energy + top-k + sparse PV ----------
            # Engine budget per 128-row tile (PE is the pacer at ~5.1us):
            #   PE : 24 matmuls -> e_psum [128,2048] f32        ~5.1us
            #   ACT: e_psum -> e_sb copy (frees PSUM fast)      ~1.7us
            #        negmax, exp8+accum, w=ex*rden, t0,t1 muls  ~1.9us
            #   DVE: max8 + max_index on e_sb, recip, final add ~5.0us
            #   SP : jd bounce x3 + y out-DMA
            #   gpsimd: dma_gather prep
            with tc.tile_pool(name="ep", bufs=2, space="PSUM") as epp, \
                 tc.tile_pool(name="es", bufs=2) as esp, \
                 tc.tile_pool(name="st", bufs=8) as stp, \
                 tc.tile_pool(name="id", bufs=6) as idxp, \
                 tc.tile_pool(name="wk", bufs=8) as wkp, \
                 tc.tile_pool(name="gp", bufs=7) as gpp, \
                 tc.tile_pool(name="tp", bufs=7) as tpp, \
                 tc.tile_pool(name="yp", bufs=6) as ypp:

                def emit_energy_scan(i, pv):
                    # E rows i*128..i*128+127 as one 4-bank PSUM tile
                    e_t = epp.tile([128, N], F32, tag="e")
                    # d outer: the same stationary qT chunk drives all 4 jb
                    # banks before the weights change
                    for d in range(EC):
                        for jb in range(NBLK):
                            nc.tensor.matmul(
                                e_t[:, jb * 512:(jb + 1) * 512],
                                lhsT=qT[:, d, i * 128:(i + 1) * 128],
                                rhs=xT[:, jb, d, :],
                                start=(d == 0),
                                stop=(d == EC - 1),
                            )
                    e_sb = esp.tile([128, N], F32, tag="es")
                    nc.scalar.copy(e_sb[:], e_t[:, :])
                    if pv is not None:
                        emit_combine_act(*pv)
                    # stats: 0:8 top8 vals, 8:16 exp(top8-max), 16 -max,
                    # 17 denom, 18 1/denom
                    # st: 0:8 top8 vals, 8:16 exp(top8-max), 16 -max,
                    # 17 denom, 18 ln(denom), 19 -max-ln(denom)
                    # st: 0:8 top8 vals, 8:16 exp(top8-max), 16 -max,
                    # 17 denom, 18 1/denom.  Only Exp + Copy run on ACT: any
                    # second activation func (Ln etc.) thrashes the 1.28us
                    # ACT table load every tile.
                    st = stp.tile([128, 20], F32, tag="st")
                    idx8 = idxp.tile([128, 8], U16, tag="ix")
                    nc.vector.max(st[:, 0:8], e_sb[:])
                    nc.scalar.mul(st[:, 16:17], st[:, 0:1], -1.0)
                    nc.vector.max_index(idx8[:], st[:, 0:8], e_sb[:])
                    nc.scalar.activation(
                        st[:, 8:16],
                        st[:, 0:8],
                        func=mybir.ActivationFunctionType.Exp,
                        bias=st[:, 16:17],
                        scale=1.0,
                        accum_out=st[:, 17:18],
                    )
                    nc.vector.reciprocal(st[:, 18:19], st[:, 17:18])
                    w16 = wkp.tile([128, K], F32, tag="w")
                    nc.scalar.mul(w16[:], st[:, 8:8 + K], st[:, 18:19])
                    if pv is not None:
                        emit_combine_dve(*pv)
                    # index bounce: SBUF [128,K] -> HBM -> SBUF [16, K, 8]
                    # (all on the SP queue; FIFO order serializes the chain).
                    # The gather runs on SWDGE queue i%4, whose two Q7 cpus
                    # (2q, 2q+1) each read their own copy of the idxs from
                    # partitions 32q..32q+16 and 32q+16..32q+32.  CoreSim
                    # always reads partitions 0:16, so q>0 writes that too.
                    # SP-queue decongestion: only the two bounce reads
                    # ride the sync queue; the jd write joins the y DMA on
                    # the ACT queue (its dep, max_index, completes mid-tile
                    # well before ACT reaches it, so no HOL stall).
                    nc.scalar.dma_start(jd_d[i], idx8[:, 0:K])
                    nc.sync.dma_start(
                        idx_sb[0:16, i, :, :].bitcast(U16), jd_r[i])
                    nc.sync.dma_start(
                        idx_sb[16:32, i, :, :].bitcast(U16), jd_r[i])
                    return w16

                def emit_gather(i, w16):
                    g = gpp.tile([128, K, E], FP16, tag="g")
                    nc.gpsimd.dma_gather(
                        g[:, :, :], vh_d[:, :], idx_sb[:, i, :, :],
                        num_idxs=128 * K, num_idxs_reg=128 * K, elem_size=E,
                    )
                    t0 = tpp.tile([128, E], FP16, tag="t0")
                    t1 = tpp.tile([128, E], FP16, tag="t1")
                    return (i, g, w16, t0, t1)

                def emit_combine_act(i, g, w16, t0, t1):
                    nc.scalar.mul(t0[:], g[:, 0, :], w16[:, 0:1])
                    nc.scalar.mul(t1[:], g[:, 1, :], w16[:, 1:2])

                def emit_combine_dve(i, g, w16, t0, t1):
                    y16 = ypp.tile([128, E], FP16, tag="y")
                    nc.vector.tensor_add(y16[:], t0[:], t1[:])
                    # y out on the ACT queue: a y DMA on the sync queue
                    # would conservatively gate the NEXT gather prep (which
                    # waits the sync-queue DMA count) behind this tile's ADD.
                    nc.scalar.dma_start(y_r[:, i, :], y16[:])

                ws, pend = [], []
                DEFER = 5
                for i in range(NT):
                    pv = pend.pop(0) if i >= DEFER else None
                    ws.append(emit_energy_scan(i, pv))
                    pend.append(emit_gather(i, ws[i]))
                for pv in pend:
                    emit_combine_act(*pv)
                    emit_combine_dve(*pv)

    nc.finalize()
    return nc


def _get_nc():
    if "nc" not in _CACHE:
        _CACHE["nc"] = _build()
    return _CACHE["nc"]


def kernel(x, Wq, Wk, Wv, Wo, _run_kwargs=None):
    from concourse.bass_utils import run_bass_kernel_spmd

    x = np.asarray(x, dtype=np.float32)
    f = np.float32
    # fold the projections across the softmax boundary (fp32 on host):
    #   E = x (8 Wq^T Wk) x^T ; y = P (x Wv^T Wo^T)
    m = ((np.asarray(Wq, f).T * np.float32(SCALE)) @ np.asarray(Wk, f)).astype(np.float16)
    mo = (np.asarray(Wv, f).T @ np.asarray(Wo, f).T).astype(np.float16)
    # partition-major DMA layouts (contiguous per partition per chunk):
    #   m4[dc, p, c, dd] = M[c*128+p, dc*128+dd]
    #   x5[nb, p, c, nn] = x.T[c*128+p, nb*512+nn]
    m4 = np.ascontiguousarray(
        m.reshape(EC, 128, EC, 128).transpose(2, 1, 0, 3).reshape(EC, 128, E)
    )

    def x5(xb):
        return np.ascontiguousarray(
            xb.T.astype(np.float16).reshape(EC, 128, NBLK, 512)
            .transpose(2, 1, 0, 3)
        )

    nc = _get_nc()
    in_maps = [
        {
            "xT": x5(x[b]),
            "m": m4,
            "mo": mo,
        }
        for b in range(B)
    ]
    res = run_bass_kernel_spmd(nc, in_maps, list(range(B)), **(_run_kwargs or {}))
    out = np.stack([res.results[b]["y"].astype(np.float32) for b in range(B)])
    if _run_kwargs:
        _CACHE["last_results"] = res
    return np.ascontiguousarray(out, dtype=np.float32)


# revision 22
# speedup vs baseline: 1.3590x; 1.3590x over previous
"""Fused multi-head-attention (full-width variant) for 8 TRN2 NeuronCores.

Strategy: pure data-parallel over batch (B=8 -> one batch per core).

Algebraic folding (softmax is the only nonlinearity):
  E = (x Wq^T)(x Wk^T)^T * 8 = x M x^T        with M  = 8 * Wq^T Wk  (host fp32)
  y = (P (x Wv^T)) Wo^T      = P (x Mo^T)     with Mo = Wo Wv        (host fp32)
so the device only runs TWO projections (q' = x M, v' = x Mo^T), the energy
matmul against raw xT, softmax, and one PV matmul that directly produces y.

Everything fp16 (fp32 PSUM accumulation), "feature-on-partition" layouts:
  q'T  = M^T @ xT            (fp16 matmuls)
  E    = q' @ x^T            (fp16, fp32 PSUM; scale pre-folded into M)
  P    = softmax rows via ACT exp (bias=-rowmax via negated reduce, accum rowsum)
  PT   = DMA-xbar transpose of P  (fp16)
  yT   = v'^T @ PT           (fp16 out, DMA'd per 512-col block)
Host casts x to fp16 + transposes; computes M/Mo in fp32 BLAS; yT back to f32.

Pipelining: PV of block b-1 is emitted after the energy/softmax of block b, so
the PE stream never waits on the exp->transpose chain. All input DMAs stream on
the SP (sync) HWDGE queue in exact consumption order (per-core HBM read BW is
the binding resource at start); output DMAs go on the ACT (scalar) HWDGE queue;
the sync queue carries the xbar transposes with their guard DMAs.
"""
import sys

sys.path.insert(0, "/opt/trn_rl_repo")

import numpy as np

import concourse.bass as bass  # noqa: F401
import concourse.tile as tile
from concourse import bacc, mybir

F32 = mybir.dt.float32
FP16 = mybir.dt.float16
AX = mybir.AxisListType.X
MAX = mybir.AluOpType.max

B = 8
E = 768
N = 2048
EC = E // 128      # 6 feature chunks
NT = N // 128      # 16 token chunks
NBLK = N // 512    # 4 blocks of 512 tokens
SCALE = 8.0        # sqrt(head_dim); reference multiplies by it

_CACHE = {}


def _build():
    nc = bacc.Bacc("TRN2", target_bir_lowering=False, debug=False, num_devices=B)

    # xT/m come in partition-major layouts so every DMA chunk is contiguous
    # per partition (1.5-6 KB pieces -> full DMA bandwidth at kernel start):
    #   xT[nb, p, c, nn] = x.T[c*128+p, nb*512+nn]
    #   m[dc, p, c, dd]  = M[c*128+p, dc*128+dd]
    xT_d = nc.dram_tensor("xT", [NBLK, 128, EC, 512], FP16, kind="ExternalInput")
    m_d = nc.dram_tensor("m", [EC, 128, E], FP16, kind="ExternalInput")
    mo_d = nc.dram_tensor("mo", [E, E], FP16, kind="ExternalInput")
    yT_d = nc.dram_tensor("yT", [E, N], FP16, kind="ExternalOutput")
    # Tiny stats dump (every tile overwrites the same region). Its real job: a
    # plain HWDGE DMA queued before every dma_start_transpose — two xbar
    # transposes back-to-back on the sync queue with no intervening plain DMA
    # produce doubled output values (observed on HW; the plain transfer forces
    # the xbar-mode transition).
    snk_d = nc.dram_tensor("snk", [128, 8], F32, kind="ExternalOutput")

    mo_r = mo_d.rearrange("(c p) f -> p c f", p=128)
    yT_r = yT_d.rearrange("(c p) n -> p c n", p=128)

    with tile.TileContext(nc) as tc:
        with tc.tile_pool(name="xT", bufs=1) as xtp, \
             tc.tile_pool(name="qT", bufs=1) as qtp, \
             tc.tile_pool(name="vv", bufs=1) as vvp:
            xT = xtp.tile([128, NBLK, EC, 512], FP16)   # 24 KB/partition
            qT = qtp.tile([128, EC, N], FP16)   # 24   (q' transposed)
            v = vvp.tile([128, NT, E], FP16)    # 24   (v' natural)

            # ---------------- stage B: projections ----------------
            with tc.tile_pool(name="wr", bufs=1) as wrp, \
                 tc.tile_pool(name="wp", bufs=2) as wpp, \
                 tc.tile_pool(name="psb", bufs=8, space="PSUM") as psb:
                # PE warm-up during the initial input-DMA window: dummy
                # matmuls push the HAM activity window so the first real
                # matmuls run at 2.4 GHz instead of 1.2 GHz (more fillers are
                # interleaved into the first paced q'T group below).
                wrm = wrp.tile([128, 512], FP16, tag="wrm")
                # DVE memset, not gpsimd: the Q7 cores boot at ~6us and their
                # memset gated the first warm-up matmul to 7.6us; the DVE is
                # up earlier, so the PE clock ramp starts sooner.
                nc.vector.memset(wrm[:], 0.0)

                def wmm():
                    wps = psb.tile([128, 512], F32, tag="ps")
                    nc.tensor.matmul(
                        wps[:],
                        lhsT=wrm[:, 0:128],
                        rhs=wrm[:],
                        start=True,
                        stop=True,
                    )

                for _w in range(10):
                    wmm()
                m_t = wpp.tile([128, EC, E], FP16, tag="w")  # 9 x2
                mo_t = wpp.tile([128, EC, E], FP16, tag="w")
                # One queue, exact consumption order (parallel HWDGE queues
                # do NOT add early bandwidth — measured twice — they contend
                # and delay the critical chunks). m and xT-nb0 stream in small
                # interleaved chunks; with the e-outer first blocks below,
                # every arriving chunk unlocks ~6 real matmuls.
                nc.sync.dma_start(m_t[:, :, 0:128], m_d[0])
                nc.sync.dma_start(xT[:, 0, 0, :], xT_d[0][:, 0, :])
                for f in range(1, EC):
                    nc.sync.dma_start(m_t[:, :, f * 128:(f + 1) * 128], m_d[f])
                    nc.sync.dma_start(xT[:, 0, f, :], xT_d[0][:, f, :])
                nc.sync.dma_start(xT[:, 1, 0:3, :], xT_d[1][:, 0:3, :])
                nc.sync.dma_start(xT[:, 1, 3:6, :], xT_d[1][:, 3:6, :])
                nc.sync.dma_start(xT[:, 2], xT_d[2])
                nc.sync.dma_start(xT[:, 3], xT_d[3])
                nc.sync.dma_start(mo_t[:], mo_r[:])

                # q'T = M^T @ xT. First block e-outer across 6 concurrent
                # PSUM accumulation groups (paced by the arriving chunks);
                # remaining blocks f-outer (their data streams well ahead).
                ps0 = [
                    psb.tile([128, 512], F32, tag="ps", name=f"ps0_{f}")
                    for f in range(EC)
                ]
                for e in range(EC):
                    for f in range(EC):
                        nc.tensor.matmul(
                            ps0[f][:],
                            lhsT=m_t[:, e, f * 128:(f + 1) * 128],
                            rhs=xT[:, 0, e, :],
                            start=(e == 0),
                            stop=(e == EC - 1),
                        )
                for f in range(EC):
                    nc.vector.tensor_copy(qT[:, f, 0:512], ps0[f][:])
                # nb1 likewise e-outer (its xT block arrives in two halves
                # while nb0's tail is still streaming)
                ps1 = [
                    psb.tile([128, 512], F32, tag="ps", name=f"ps1_{f}")
                    for f in range(EC)
                ]
                for e in range(EC):
                    for f in range(EC):
                        nc.tensor.matmul(
                            ps1[f][:],
                            lhsT=m_t[:, e, f * 128:(f + 1) * 128],
                            rhs=xT[:, 1, e, :],
                            start=(e == 0),
                            stop=(e == EC - 1),
                        )
                for f in range(EC):
                    nc.vector.tensor_copy(qT[:, f, 512:1024], ps1[f][:])
                for nb in range(2, NBLK):
                    for f in range(EC):
                        ps = psb.tile([128, 512], F32, tag="ps")
                        for e in range(EC):
                            nc.tensor.matmul(
                                ps[:],
                                lhsT=m_t[:, e, f * 128:(f + 1) * 128],
                                rhs=xT[:, nb, e, :],
                                start=(e == 0),
                                stop=(e == EC - 1),
                            )
                        nc.vector.tensor_copy(qT[:, f, nb * 512:(nb + 1) * 512], ps[:])

                # v' (natural layout) = x @ Mo^T
                for t in range(NT):
                    for flo, fhi in ((0, 512), (512, 768)):
                        ps = psb.tile([128, 512], F32, tag="ps")
                        for e in range(EC):
                            nc.tensor.matmul(
                                ps[:, :fhi - flo],
                                lhsT=xT[:, t // 4, e, (t % 4) * 128:(t % 4 + 1) * 128],
                                rhs=mo_t[:, e, flo:fhi],
                                start=(e == 0),
                                stop=(e == EC - 1),
                            )
                        nc.scalar.copy(v[:, t, flo:fhi], ps[:, :fhi - flo])

            # ---------------- stage C/D/E: attention ----------------
            with tc.tile_pool(name="pp", bufs=4) as ppp, \
                 tc.tile_pool(name="pt", bufs=2) as ptp, \
                 tc.tile_pool(name="yt", bufs=4) as ytp, \
                 tc.tile_pool(name="st", bufs=6) as stp, \
                 tc.tile_pool(name="pse", bufs=6, space="PSUM") as pse, \
                 tc.tile_pool(name="psm", bufs=2, space="PSUM") as psm:

                def emit_energy_softmax(ib, pt_blk):
                    for t4 in range(4):
                        i = ib * 4 + t4
                        # one tile for all per-row stats: cols 0-3 jb-maxes,
                        # 4-7 jb-expsums, 8 -rowmax, 9 rowsum, 10 1/rowsum
                        stats = stp.tile([128, 12], F32, tag="stats")
                        e_tiles = []
                        for jb in range(NBLK):
                            pe = pse.tile([128, 512], F32)
                            for d in range(EC):
                                nc.tensor.matmul(
                                    pe[:],
                                    lhsT=qT[:, d, i * 128:(i + 1) * 128],
                                    rhs=xT[:, jb, d, :],
                                    start=(d == 0),
                                    stop=(d == EC - 1),
                                )
                            nc.vector.tensor_reduce(
                                stats[:, jb:jb + 1], pe[:], axis=AX, op=MAX
                            )
                            e_tiles.append(pe)
                        nmax = stats[:, 8:9]
                        nc.vector.tensor_reduce(
                            nmax, stats[:, 0:4], axis=AX, op=MAX, negate=True)

                        p_t = ppp.tile([128, N], FP16)  # 4 x4
                        for jb in range(NBLK):
                            nc.scalar.activation(
                                p_t[:, jb * 512:(jb + 1) * 512],
                                e_tiles[jb][:],
                                func=mybir.ActivationFunctionType.Exp,
                                bias=nmax,
                                scale=1.0,
                                accum_out=stats[:, 4 + jb:5 + jb],
                            )
                        nc.vector.tensor_reduce(
                            stats[:, 9:10], stats[:, 4:8], axis=AX,
                            op=mybir.AluOpType.add
                        )
                        nc.vector.reciprocal(stats[:, 10:11], stats[:, 9:10])
                        nc.vector.tensor_scalar_mul(p_t[:], p_t[:], stats[:, 10:11])
                        # ALL transposes on one HWDGE queue, each preceded by
                        # a plain guard DMA: concurrent xbar transposes (even on
                        # different queues) corrupt results — xbar state is
                        # per-core global
                        nc.sync.dma_start(snk_d[:], stats[:, 0:8])
                        nc.sync.dma_start_transpose(
                            pt_blk[:, :, t4 * 128:(t4 + 1) * 128], p_t[:]
                        )

                def emit_pv(ib, pt_blk, halves=False):
                    # yT block = v'^T @ PT, 512-wide. (With PV running a full
                    # block behind the energy/softmax stage, even the trailing
                    # block's transposes complete long before its PV starts,
                    # so no half-split is needed anywhere.)
                    for lo, hi in ((0, 256), (256, 512)) if halves else ((0, 512),):
                        for f in range(EC):
                            po = psm.tile([128, 512], F32, tag="mm")
                            for jc in range(NT):
                                nc.tensor.matmul(
                                    po[:, :hi - lo],
                                    lhsT=v[:, jc, f * 128:(f + 1) * 128],
                                    rhs=pt_blk[:, jc, lo:hi],
                                    start=(jc == 0),
                                    stop=(jc == NT - 1),
                                )
                            yt = ytp.tile([128, 512], FP16)
                            nc.vector.tensor_copy(yt[:, :hi - lo], po[:, :hi - lo])
                            nc.scalar.dma_start(
                                yT_r[:, f, ib * 512 + lo:ib * 512 + hi],
                                yt[:, :hi - lo],
                            )

                pt_blks = []
                for ib in range(NBLK):
                    pt_blk = ptp.tile([128, NT, 512], FP16)  # 16 x2
                    pt_blks.append(pt_blk)
                    emit_energy_softmax(ib, pt_blk)
                    if ib > 0:
                        emit_pv(ib - 1, pt_blks[ib - 1])
                emit_pv(NBLK - 1, pt_blks[NBLK - 1])

    nc.finalize()
    return nc


def _get_nc():
    if "nc" not in _CACHE:
        _CACHE["nc"] = _build()
    return _CACHE["nc"]


def kernel(x, Wq, Wk, Wv, Wo, _run_kwargs=None):
    from concourse.bass_utils import run_bass_kernel_spmd

    x = np.asarray(x, dtype=np.float32)
    f = np.float32
    # fold the projections across the softmax boundary (fp32 on host):
    #   E = x (8 Wq^T Wk) x^T ; y = P (x (Wo Wv)^T)
    m = ((np.asarray(Wq, f).T * np.float32(SCALE)) @ np.asarray(Wk, f)).astype(np.float16)
    mo = (np.asarray(Wv, f).T @ np.asarray(Wo, f).T).astype(np.float16)
    # partition-major DMA layouts (contiguous per partition per chunk):
    #   m4[dc, p, c, dd] = M[c*128+p, dc*128+dd]
    #   x5[nb, p, c, nn] = x.T[c*128+p, nb*512+nn]
    m4 = np.ascontiguousarray(
        m.reshape(EC, 128, EC, 128).transpose(2, 1, 0, 3).reshape(EC, 128, E)
    )

    def x5(xb):
        return np.ascontiguousarray(
            xb.T.astype(np.float16).reshape(EC, 128, NBLK, 512)
            .transpose(2, 1, 0, 3)
        )

    nc = _get_nc()
    in_maps = [
        {
            "xT": x5(x[b]),
            "m": m4,
            "mo": mo,
        }
        for b in range(B)
    ]
    res = run_bass_kernel_spmd(nc, in_maps, list(range(B)), **(_run_kwargs or {}))
    out = np.stack([res.results[b]["yT"].T.astype(np.float32) for b in range(B)])
    if _run_kwargs:
        _CACHE["last_results"] = res
    return np.ascontiguousarray(out, dtype=np.float32)

